# revision 52
# baseline (speedup 1.0000x reference)
"""Trainium2 Bass kernel for nn_CombinedLoss (deep-supervision CE + spectrum loss).

Strategy (pure data parallel over batch B=512 -> 64 rows on each of 8 cores):

CE part (per core):
  logits [6,64,40,28] -> SBUF [128, 6,20,28] (partition = (b, s-half)).
  e3 = exp(logits); se = sum_v e3; lse = ln(se)   (no max-sub needed: |x|<~6)
  one-hot(targets) built via iota + is_equal; x_t = sum_v logits*onehot (fused
  multiply+accumulate per t); ce partial = sum w_t * mask * (lse - x_t).

Spectrum part (per core):
  probs(t=5) -> expected residue mass -> cumsum via PE matmul with an
  upper-triangular ones matrix -> theoretical ion masses theo [64, 111].
  Observed peaks are host-side compacted (masked peaks moved to a 1e9 tail, so
  the array stays sorted) => for each ion only a narrow window of peaks can be
  within +-0.5 Da.  Window start found by coarse searchsorted (count of every
  8th peak below theo-0.51, via fused compare+accumulate split over the DVE and
  ACT engines), then one indirect DMA gathers 16 (mass,intensity) pairs per ion.
  The softmax/huber/intensity sums then run on the compact [128, 56*16] tiles.

Each core returns per-partition partial sums [128,4] = (ce_num, mask_cnt,
spec_num, spec_cnt); the host reduces and combines the final scalar.
"""

import numpy as np

T, B, S, V = 6, 512, 40, 28
N_PEAKS = 512
NCORES = 8
BS = B // NCORES          # 64 batch rows per core
PROTON = 1.007276
WATER = 18.010565
CO = 27.994915
MASS_TOL = 0.5
TEMP = 0.1
HUB_D = 0.2
CE_W = 1.0
SPEC_W = 0.1

NRES = S - 2              # 38 residues
NI = S - 3                # 37 ions per family
P_IONS = 3 * NI           # 111
NPAIR = 56                # ceil(111/2) ion pairs (one per S1 iteration)
NPAD = 528                # padded peak count (gather overrun safety)
W_GATH = 12               # gathered window width (peaks per ion)
COARSE = 8                # coarse searchsorted stride
NCOARSE = N_PEAKS // COARSE   # 64
SLACK = 0.01              # extra margin below theo-0.5 for the window start
BIG = 1.0e9               # sentinel mass for masked / padded peaks
N_ACT_S1 = 18             # S1 iterations that run on the ACT engine (rest DVE)

_cached = {}


def _build_program():
    import concourse.bass as bass
    import concourse.bacc as bacc
    import concourse.mybir as mybir
    import concourse.tile as tile
    from concourse.masks import make_identity, make_upper_triangular

    dt = mybir.dt
    Alu = mybir.AluOpType
    Act = mybir.ActivationFunctionType
    AX = mybir.AxisListType

    nc = bacc.Bacc("TRN2", target_bir_lowering=False, debug=False,
                   num_devices=NCORES)

    lg_d = nc.dram_tensor("logits", [T, BS, S, V], dt.float32, kind="ExternalInput")
    tgt_d = nc.dram_tensor("targets", [BS, S], dt.int32, kind="ExternalInput")
    tm_d = nc.dram_tensor("tmask", [BS, S], dt.uint8, kind="ExternalInput")
    obs_d = nc.dram_tensor("obseff", [BS, N_PEAKS], dt.float32, kind="ExternalInput")
    # pairs rows are (obs,int) interleaved, 1056 f32 = 33*32; the [.., 32]
    # last dim matches one gathered window (W_GATH*2 = 32 elems = 128 B) so
    # descriptor accounting sees per-index 128 B transfers.
    pairs_d = nc.dram_tensor("pairs", [BS, 33, 32], dt.float32, kind="ExternalInput")
    aa_d = nc.dram_tensor("aa128", [128, V], dt.float32, kind="ExternalInput")
    out_d = nc.dram_tensor("partials", [128, 4], dt.float32, kind="ExternalOutput")

    f32 = dt.float32

    with tile.TileContext(nc) as tc:
        with tc.tile_pool(name="main", bufs=1) as pool, \
             tc.tile_pool(name="dram", bufs=1, space="DRAM") as dram_pool, \
             tc.tile_pool(name="ps", bufs=1, space="PSUM") as psp:

            # ---------------- input DMAs ----------------
            lg = pool.tile([128, T, 20, V], f32, tag="lg")
            # DRAM addr of [t, b, s, v] = t*B*S*V + (p*20+q)*V + v with
            # p = 2b + s//20, q = s%20 -> per-t uniform-stride partition dim.
            lg_src = (lg_d.ap().rearrange("t b s v -> t (b s) v")
                      .rearrange("t (p q) v -> t p (q v)", p=128))
            # t=5 first: the spectrum path (critical) only needs logits[5]
            for t in (5, 0, 1, 2, 3, 4):
                nc.sync.dma_start(out=lg[:, t].rearrange("p j v -> p (j v)"),
                                  in_=lg_src[t])

            tgt_i = pool.tile([128, 20], dt.int32, tag="tgt_i")
            nc.sync.dma_start(out=tgt_i[:],
                              in_=tgt_d.ap().rearrange("b (h j) -> (b h) j", h=2))
            tm_u = pool.tile([128, 20], dt.uint8, tag="tm_u")
            nc.sync.dma_start(out=tm_u[:],
                              in_=tm_d.ap().rearrange("b (h j) -> (b h) j", h=2))

            obs2 = pool.tile([128, N_PEAKS], f32, tag="obs2")
            nc.sync.dma_start(
                out=obs2[:],
                in_=obs_d.ap()[None].broadcast_to([2, BS, N_PEAKS]))

            aar = pool.tile([128, V], f32, tag="aar")
            nc.sync.dma_start(out=aar[:], in_=aa_d.ap())

            # ---------------- CE part ----------------
            e3 = pool.tile([128, T, 20, V], f32, tag="e3")
            se = pool.tile([128, T, 20], f32, tag="se")
            for t in (5, 0, 1, 2, 3, 4):
                nc.scalar.activation(out=e3[:, t], in_=lg[:, t], func=Act.Exp)
                nc.vector.tensor_reduce(out=se[:, t], in_=e3[:, t],
                                        axis=AX.X, op=Alu.add)

            # masks
            mf = pool.tile([128, 20], f32, tag="mf")
            nc.vector.tensor_copy(out=mf[:], in_=tm_u[:])
            tgtf = pool.tile([128, 20], f32, tag="tgtf")
            nc.vector.tensor_copy(out=tgtf[:], in_=tgt_i[:])
            nz = pool.tile([128, 20], f32, tag="nz")
            nc.vector.tensor_scalar(out=nz[:], in0=tgtf[:], scalar1=0.5,
                                    scalar2=None, op0=Alu.is_gt)
            Mm = pool.tile([128, 20], f32, tag="Mm")
            nc.vector.tensor_tensor(out=Mm[:], in0=mf[:], in1=nz[:], op=Alu.mult)

            partials = pool.tile([128, 4], f32, tag="partials")
            # mask count partial
            nc.vector.tensor_reduce(out=partials[:, 1:2], in_=mf[:],
                                    axis=AX.X, op=Alu.add)

            # (one-hot / xtm / w-weights are emitted after S3: their DVE work
            # is off the critical path and fills the gather gap)

            # ---------------- spectrum: theo masses ----------------
            # probs(t=5) expected residue mass, partition layout [128, 20]
            pe5 = pool.tile([128, 20], f32, tag="pe5")
            nc.vector.reciprocal(out=pe5[:], in_=se[:, 5])
            prod5 = pool.tile([128, 20, V], f32, tag="prod5")
            nc.vector.tensor_tensor(out=prod5[:], in0=e3[:, 5],
                                    in1=aar[:, None, :].broadcast_to([128, 20, V]),
                                    op=Alu.mult)
            nume = pool.tile([128, 20], f32, tag="nume")
            nc.vector.tensor_reduce(out=nume[:], in_=prod5[:], axis=AX.X, op=Alu.add)
            expected = pool.tile([128, 20], f32, tag="expected")
            nc.vector.tensor_tensor(out=expected[:], in0=nume[:], in1=pe5[:],
                                    op=Alu.mult)

            # reshape to [64, 40] (partition = b) via 2 SBUF->SBUF DMAs
            # selector matmuls: exp64[b, h*20+j] = expected[2b+h, j]
            iota_2b = pool.tile([128, 64], dt.int32, tag="iota_2b")
            nc.gpsimd.iota(iota_2b[:], pattern=[[2, 64]], channel_multiplier=0)
            iota_pp = pool.tile([128, 1], dt.int32, tag="iota_pp")
            nc.gpsimd.iota(iota_pp[:], pattern=[[0, 1]], channel_multiplier=1)
            i2b_f = pool.tile([128, 64], f32, tag="i2b_f")
            nc.vector.tensor_copy(out=i2b_f[:], in_=iota_2b[:])
            pp_f = pool.tile([128, 1], f32, tag="pp_f")
            nc.vector.tensor_copy(out=pp_f[:], in_=iota_pp[:])
            pm1_f = pool.tile([128, 1], f32, tag="pm1_f")
            nc.vector.tensor_scalar(out=pm1_f[:], in0=pp_f[:], scalar1=1.0,
                                    scalar2=None, op0=Alu.subtract)
            selh0 = pool.tile([128, 64], f32, tag="selh0")
            nc.vector.tensor_scalar(out=selh0[:], in0=i2b_f[:], scalar1=pp_f[:],
                                    scalar2=None, op0=Alu.is_equal)
            selh1 = pool.tile([128, 64], f32, tag="selh1")
            nc.vector.tensor_scalar(out=selh1[:], in0=i2b_f[:], scalar1=pm1_f[:],
                                    scalar2=None, op0=Alu.is_equal)

            exp64_ps = psp.tile([64, S], f32, tag="exp64_ps")
            nc.tensor.matmul(out=exp64_ps[:, 0:20], lhsT=selh0[:],
                             rhs=expected[:], start=True, stop=True)
            nc.tensor.matmul(out=exp64_ps[:, 20:40], lhsT=selh1[:],
                             rhs=expected[:], start=True, stop=True)
            exp64 = pool.tile([64, S], f32, tag="exp64")
            nc.vector.tensor_copy(out=exp64[:], in_=exp64_ps[:])

            # transpose -> [38, 64] via PE
            ident = pool.tile([64, 64], f32, tag="ident")
            make_identity(nc, ident[:])
            expT_ps = psp.tile([NRES, 64], f32, tag="expT_ps")
            nc.tensor.transpose(out=expT_ps[:], in_=exp64[:, 1:1 + NRES],
                                identity=ident[:])
            expT = pool.tile([NRES, 64], f32, tag="expT")
            nc.vector.tensor_copy(out=expT[:], in_=expT_ps[:])

            # cumsum over residues via matmul with upper-triangular ones
            ut = pool.tile([NRES, NRES], f32, tag="ut")
            make_upper_triangular(nc, ut[:], val=1.0)
            cum_ps = psp.tile([64, NRES], f32, tag="cum_ps")
            nc.tensor.matmul(out=cum_ps[:], lhsT=expT[:], rhs=ut[:],
                             start=True, stop=True)
            cum = pool.tile([64, NRES], f32, tag="cum")
            nc.vector.tensor_copy(out=cum[:], in_=cum_ps[:])

            lastWP = pool.tile([64, 1], f32, tag="lastWP")
            nc.vector.tensor_scalar(out=lastWP[:], in0=cum[:, NRES - 1:NRES],
                                    scalar1=WATER + PROTON, scalar2=None,
                                    op0=Alu.add)

            theo = pool.tile([64, 112], f32, tag="theo")
            nc.vector.tensor_scalar(out=theo[:, 0:37], in0=cum[:, 0:37],
                                    scalar1=PROTON, scalar2=None, op0=Alu.add)
            nc.vector.tensor_copy(out=theo[:, 37:38], in_=lastWP[:])
            nc.vector.tensor_scalar(out=theo[:, 38:74], in0=cum[:, 0:36],
                                    scalar1=-1.0, scalar2=lastWP[:],
                                    op0=Alu.mult, op1=Alu.add)
            nc.vector.tensor_scalar(out=theo[:, 74:111], in0=cum[:, 0:37],
                                    scalar1=PROTON - CO, scalar2=None, op0=Alu.add)
            nc.vector.memset(theo[:, 111:112], -BIG)

            # duplicate across partition halves, split even/odd ions
            # duplicate theo to both partition halves via PE selector:
            # sel[b, p] = 1 iff (p & 63) == b
            iota_q = pool.tile([64, 2, 64], dt.int32, tag="iota_q")
            nc.gpsimd.iota(iota_q[:], pattern=[[0, 2], [1, 64]],
                           channel_multiplier=0)
            iq_f = pool.tile([64, 2, 64], f32, tag="iq_f")
            nc.vector.tensor_copy(out=iq_f[:], in_=iota_q[:])
            sel128 = pool.tile([64, 128], f32, tag="sel128")
            nc.vector.tensor_scalar(out=sel128[:],
                                    in0=iq_f[:].rearrange("p a b -> p (a b)"),
                                    scalar1=pp_f[0:64], scalar2=None,
                                    op0=Alu.is_equal)
            theo2_ps = psp.tile([128, 112], f32, tag="theo2_ps")
            nc.tensor.matmul(out=theo2_ps[:], lhsT=sel128[:], rhs=theo[:],
                             start=True, stop=True)
            theo2 = pool.tile([128, 112], f32, tag="theo2")
            nc.vector.tensor_copy(out=theo2[:], in_=theo2_ps[:])
            theo_v = theo2[:].rearrange("p (i two) -> p i two", two=2)
            theo_stk = pool.tile([128, NPAIR], f32, tag="theo_stk")
            nc.vector.tensor_copy(out=theo_stk[0:64], in_=theo_v[0:64, :, 0])
            nc.vector.tensor_copy(out=theo_stk[64:128], in_=theo_v[64:128, :, 1])

            thr = pool.tile([128, NPAIR], f32, tag="thr")
            nc.vector.tensor_scalar(out=thr[:], in0=theo_stk[:],
                                    scalar1=MASS_TOL + SLACK,
                                    scalar2=None, op0=Alu.subtract)

            # ---------------- S1: coarse searchsorted (batched) ----------------
            obsc = pool.tile([128, NCOARSE], f32, tag="obsc")
            nc.vector.tensor_copy(
                out=obsc[:],
                in_=obs2[:].rearrange("p (c e) -> p c e", e=COARSE)[:, :, 0])

            # window starts: elem offset = 2*max(8*lo8 - 8, 0) + b*2*NPAD
            iota_p = pool.tile([128, 1], dt.int32, tag="iota_p")
            nc.gpsimd.iota(iota_p[:], pattern=[[0, 1]], channel_multiplier=2 * NPAD)
            pb_f = pool.tile([128, 1], f32, tag="pb_f")
            nc.vector.tensor_copy(out=pb_f[:], in_=iota_p[:])
            base_f = pool.tile([128, 1], f32, tag="base_f")
            nc.vector.tensor_copy(out=base_f[0:64], in_=pb_f[0:64])
            nc.vector.tensor_scalar(out=base_f[64:128], in0=pb_f[64:128],
                                    scalar1=-float(64 * 2 * NPAD), scalar2=None,
                                    op0=Alu.add)

            HALF = NPAIR // 2
            lo8 = pool.tile([128, NPAIR], f32, tag="lo8")
            cmp3 = pool.tile([128, HALF, NCOARSE], f32, tag="cmp3")
            st0 = pool.tile([128, NPAIR], f32, tag="st0")
            st1 = pool.tile([128, NPAIR], f32, tag="st1")
            off_u = pool.tile([128, NPAIR], dt.uint32, tag="off_u")
            cmpt = pool.tile([128, NPAIR, W_GATH, 2], f32, tag="cmpt")
            for h0 in range(2):
                sl = slice(h0 * HALF, (h0 + 1) * HALF)
                cmp_t = cmp3[:] if h0 == 0 else \
                    pool.tile([128, HALF, NCOARSE], f32, tag="cmp3b")
                cmp_eng = nc.vector
                cmp_eng.tensor_tensor(
                    out=cmp_t,
                    in0=obsc[:, None, :].broadcast_to([128, HALF, NCOARSE]),
                    in1=thr[:, sl][:, :, None].broadcast_to([128, HALF, NCOARSE]),
                    op=Alu.is_lt)
                nc.vector.tensor_reduce(out=lo8[:, sl], in_=cmp_t,
                                        axis=AX.X, op=Alu.add)
                nc.vector.tensor_scalar(out=st0[:, sl], in0=lo8[:, sl],
                                        scalar1=float(2 * COARSE),
                                        scalar2=-float(2 * COARSE), op0=Alu.mult,
                                        op1=Alu.add)
                nc.vector.tensor_scalar(out=st1[:, sl], in0=st0[:, sl],
                                        scalar1=0.0, scalar2=base_f[:],
                                        op0=Alu.max, op1=Alu.add)
                nc.vector.tensor_copy(out=off_u[:, sl], in_=st1[:, sl])
                # S2: gather this half's windows
                g = nc.gpsimd.indirect_dma_start(
                    out=cmpt[:, sl].rearrange("p a b c -> p (a b c)"),
                    out_offset=None,
                    in_=pairs_d.ap(),
                    in_offset=bass.IndirectOffsetOnAxis(ap=off_u[:, sl], axis=2))
                if h0 == 0:
                    gather1 = g

            # ---------------- S3: compact windowed softmax ----------------
            og = cmpt[:, :, :, 0]
            ig = cmpt[:, :, :, 1]
            theoB = theo_stk[:, :, None].broadcast_to([128, NPAIR, W_GATH])

            d0 = pool.tile([128, NPAIR, W_GATH], f32, tag="d0")
            nc.vector.tensor_tensor(out=d0[:], in0=og, in1=theoB, op=Alu.subtract)
            dd = pool.tile([128, NPAIR, W_GATH], f32, tag="dd")
            nc.vector.scalar_tensor_tensor(out=dd[:], in0=d0[:], scalar=-1.0,
                                           in1=d0[:], op0=Alu.mult, op1=Alu.max)
            ee = pool.tile([128, NPAIR, W_GATH], f32, tag="ee")
            nc.scalar.activation(out=ee[:], in_=dd[:], func=Act.Exp,
                                 scale=-1.0 / TEMP)
            ew = pool.tile([128, NPAIR, W_GATH], f32, tag="ew")
            nc.vector.scalar_tensor_tensor(out=ew[:], in0=dd[:], scalar=MASS_TOL,
                                           in1=ee[:], op0=Alu.is_lt, op1=Alu.mult)
            den = pool.tile([128, NPAIR], f32, tag="den")
            nc.vector.tensor_reduce(out=den[:], in_=ew[:], axis=AX.X, op=Alu.add)

            c1 = pool.tile([128, NPAIR, W_GATH], f32, tag="c1")
            nc.vector.tensor_scalar(out=c1[:], in0=dd[:], scalar1=HUB_D,
                                    scalar2=float(np.sqrt(0.5)),
                                    op0=Alu.min, op1=Alu.mult)
            hm = pool.tile([128, NPAIR, W_GATH], f32, tag="hm")
            nc.scalar.activation(out=hm[:], in_=c1[:], func=Act.Square)
            rbias = pool.tile([128, 1], f32, tag="rbias")
            nc.vector.memset(rbias[:], -HUB_D * HUB_D)
            rr = pool.tile([128, NPAIR, W_GATH], f32, tag="rr")
            nc.scalar.activation(out=rr[:], in_=dd[:], func=Act.Relu,
                                 scale=HUB_D, bias=rbias[:])
            hub = pool.tile([128, NPAIR, W_GATH], f32, tag="hub")
            nc.vector.scalar_tensor_tensor(out=hub[:], in0=rr[:],
                                           scalar=HUB_D * (MASS_TOL - HUB_D),
                                           in1=hm[:], op0=Alu.min, op1=Alu.add)
            # offload the two element-wise products to the idle GPSIMD engine
            he = pool.tile([128, NPAIR, W_GATH], f32, tag="he")
            nc.gpsimd.tensor_tensor(out=he[:], in0=ew[:], in1=hub[:], op=Alu.mult)
            hubnum = pool.tile([128, NPAIR], f32, tag="hubnum")
            nc.vector.tensor_reduce(out=hubnum[:], in_=he[:], axis=AX.X, op=Alu.add)
            ie = pool.tile([128, NPAIR, W_GATH], f32, tag="ie")
            nc.gpsimd.tensor_tensor(out=ie[:], in0=ew[:], in1=ig, op=Alu.mult)
            iwnum = pool.tile([128, NPAIR], f32, tag="iwnum")
            nc.vector.tensor_reduce(out=iwnum[:], in_=ie[:], axis=AX.X, op=Alu.add)

            # ---------------- S4: per-ion contributions ----------------
            nm = pool.tile([128, NPAIR], f32, tag="nm")
            nc.vector.tensor_scalar(out=nm[:], in0=den[:], scalar1=0.0,
                                    scalar2=None, op0=Alu.is_gt)
            dsafe = pool.tile([128, NPAIR], f32, tag="dsafe")
            nc.vector.tensor_scalar(out=dsafe[:], in0=den[:], scalar1=1e-20,
                                    scalar2=None, op0=Alu.max)
            rec = pool.tile([128, NPAIR], f32, tag="rec")
            nc.vector.reciprocal(out=rec[:], in_=dsafe[:])
            t1 = pool.tile([128, NPAIR], f32, tag="t1")
            nc.vector.tensor_tensor(out=t1[:], in0=hubnum[:], in1=iwnum[:],
                                    op=Alu.mult)
            t2 = pool.tile([128, NPAIR], f32, tag="t2")
            nc.vector.tensor_tensor(out=t2[:], in0=t1[:], in1=rec[:], op=Alu.mult)
            t3 = pool.tile([128, NPAIR], f32, tag="t3")
            nc.vector.tensor_tensor(out=t3[:], in0=t2[:], in1=rec[:], op=Alu.mult)
            junk56 = pool.tile([128, NPAIR], f32, tag="junk56")
            nc.vector.scalar_tensor_tensor(
                out=junk56[:], in0=t3[:], scalar=1.0, in1=nm[:],
                op0=Alu.mult, op1=Alu.mult, accum_out=partials[:, 2:3])
            nc.vector.tensor_reduce(out=partials[:, 3:4], in_=nm[:],
                                    axis=AX.X, op=Alu.add)

            # ---------------- CE one-hot / xtm (fills the gather gap) --------
            from concourse.tile import add_dep_helper
            iota_v = pool.tile([128, 20, V], dt.int32, tag="iota_v")
            nc.gpsimd.iota(iota_v[:], pattern=[[0, 20], [1, V]],
                           channel_multiplier=0)
            oh = pool.tile([128, 20, V], f32, tag="oh")
            i_oh = nc.vector.tensor_tensor(
                out=oh[:], in0=iota_v[:],
                in1=tgt_i[:, :, None].broadcast_to([128, 20, V]),
                op=Alu.is_equal)
            add_dep_helper(i_oh.ins, gather1.ins, sync=False,
                           reason="fill gather gap")
            ohm = pool.tile([128, 20, V], f32, tag="ohm")
            nc.vector.tensor_tensor(out=ohm[:], in0=oh[:],
                                    in1=Mm[:, :, None].broadcast_to([128, 20, V]),
                                    op=Alu.mult)
            xtm = pool.tile([128, T], f32, tag="xtm")
            junk560 = pool.tile([128, 20, V], f32, tag="junk560")
            for t in range(T):
                nc.vector.scalar_tensor_tensor(
                    out=junk560[:].rearrange("p a b -> p (a b)"),
                    in0=lg[:, t].rearrange("p a b -> p (a b)"),
                    scalar=1.0,
                    in1=ohm[:].rearrange("p a b -> p (a b)"),
                    op0=Alu.mult, op1=Alu.mult,
                    accum_out=xtm[:, t:t + 1])
            iota_t = pool.tile([128, T], dt.int32, tag="iota_t")
            nc.gpsimd.iota(iota_t[:], pattern=[[1, T]], channel_multiplier=0)
            wsf = pool.tile([128, T], f32, tag="wsf")
            i_wsf = nc.vector.tensor_copy(out=wsf[:], in_=iota_t[:])
            add_dep_helper(i_wsf.ins, gather1.ins, sync=False,
                           reason="fill gather gap")
            ws = pool.tile([128, T], f32, tag="ws")
            nc.vector.tensor_scalar(out=ws[:], in0=wsf[:], scalar1=1.0,
                                    scalar2=1.0 / 21.0, op0=Alu.add, op1=Alu.mult)
            wM = pool.tile([128, T, 20], f32, tag="wM")
            nc.vector.tensor_tensor(out=wM[:],
                                    in0=Mm[:, None, :].broadcast_to([128, T, 20]),
                                    in1=ws[:, :, None].broadcast_to([128, T, 20]),
                                    op=Alu.mult)

            # ---------------- CE tail (deferred: Ln after S3's Exp) ----------
            lse = pool.tile([128, T, 20], f32, tag="lse")
            nc.scalar.activation(out=lse.rearrange("p a b -> p (a b)"),
                                 in_=se.rearrange("p a b -> p (a b)"),
                                 func=Act.Ln)
            ce1 = pool.tile([128, 1], f32, tag="ce1")
            junk120 = pool.tile([128, T, 20], f32, tag="junk120")
            nc.vector.scalar_tensor_tensor(
                out=junk120[:].rearrange("p a b -> p (a b)"),
                in0=lse[:].rearrange("p a b -> p (a b)"),
                scalar=1.0,
                in1=wM[:].rearrange("p a b -> p (a b)"),
                op0=Alu.mult, op1=Alu.mult, accum_out=ce1[:])
            ce2 = pool.tile([128, 1], f32, tag="ce2")
            junk6 = pool.tile([128, T], f32, tag="junk6")
            nc.vector.scalar_tensor_tensor(
                out=junk6[:], in0=xtm[:], scalar=1.0, in1=ws[:],
                op0=Alu.mult, op1=Alu.mult, accum_out=ce2[:])
            nc.vector.scalar_tensor_tensor(out=partials[:, 0:1], in0=ce2[:],
                                           scalar=-1.0, in1=ce1[:],
                                           op0=Alu.mult, op1=Alu.add)

            # ---------------- output ----------------
            nc.sync.dma_start(out=out_d.ap(), in_=partials[:])

    nc.compile()
    return nc


def _get_nc():
    if "nc" not in _cached:
        _cached["nc"] = _build_program()
    return _cached["nc"]


def _host_prep(all_logits, targets, target_mask, observed_masses,
               observed_intensities, peak_mask, aa_masses):
    """Shard + preprocess inputs into per-core input maps."""
    all_logits = np.ascontiguousarray(all_logits, dtype=np.float32)
    targets = np.ascontiguousarray(targets, dtype=np.int32)
    tmask = np.ascontiguousarray(target_mask).astype(np.uint8)
    obs = np.asarray(observed_masses, dtype=np.float32)
    inten = np.asarray(observed_intensities, dtype=np.float32)
    pmask = np.asarray(peak_mask)
    aa = np.asarray(aa_masses, dtype=np.float32)

    # compact masked peaks to a 1e9 tail (order within unmasked preserved
    # since obs rows are sorted; sums are permutation invariant)
    key = np.where(pmask, obs, np.inf)
    order = np.argsort(key, axis=-1, kind="stable")
    obs_eff = np.take_along_axis(np.where(pmask, obs, BIG).astype(np.float32),
                                 order, axis=-1)
    int_eff = np.take_along_axis(inten, order, axis=-1)

    pairs = np.empty((B, NPAD, 2), dtype=np.float32)
    pairs[:, :N_PEAKS, 0] = obs_eff
    pairs[:, N_PEAKS:, 0] = BIG
    pairs[:, :N_PEAKS, 1] = int_eff
    pairs[:, N_PEAKS:, 1] = 0.0
    pairs = pairs.reshape(B, 33, 32)

    aa128 = np.ascontiguousarray(np.broadcast_to(aa[None, :], (128, V)),
                                 dtype=np.float32)

    in_maps = []
    for c in range(NCORES):
        sl = slice(c * BS, (c + 1) * BS)
        in_maps.append({
            "logits": np.ascontiguousarray(all_logits[:, sl]),
            "targets": np.ascontiguousarray(targets[sl]),
            "tmask": np.ascontiguousarray(tmask[sl]),
            "obseff": np.ascontiguousarray(obs_eff[sl]),
            "pairs": np.ascontiguousarray(pairs[sl]),
            "aa128": aa128,
        })
    return in_maps


def _combine(results):
    ce_num = 0.0
    mf_cnt = 0.0
    sp_num = 0.0
    sp_cnt = 0.0
    for r in results:
        p = r["partials"].astype(np.float64)
        ce_num += p[:, 0].sum()
        mf_cnt += p[:, 1].sum()
        sp_num += p[:, 2].sum()
        sp_cnt += p[:, 3].sum()
    ce = ce_num / max(mf_cnt, 1.0)
    spec = sp_num / max(sp_cnt, 1.0)
    return np.float32(CE_W * ce + SPEC_W * spec)


def kernel(**inputs) -> np.ndarray:
    from concourse.bass_utils import run_bass_kernel_spmd

    nc = _get_nc()
    in_maps = _host_prep(**inputs)
    res = run_bass_kernel_spmd(nc, in_maps, core_ids=list(range(NCORES)))
    return _combine(res.results)
